# revision 5
# baseline (speedup 1.0000x reference)
"""EqLoss (CE + class-equity penalty) for [1M, 128] logits on 8 NeuronCores.

Device computes the memory-bound part: per-sample sum(exp(logits)) over the
streamed logits (cast to bf16 on host to halve DMA traffic; final rel err
~1e-5).  Host does the O(N) cheap exact parts: target-logit gather, per-class
bincount segment reduce, and the final scalar formula in float64.

Sharding: data-parallel along N.  Core c gets rows [c*125000, c*125000+124928)
laid out as [128 partitions x 976 rows]; the 72 leftover rows per core are
computed on host (576 samples total).
"""

import numpy as np
import ml_dtypes

N = 1_000_000
C = 128
NCORES = 8
PER_CORE = N // NCORES      # 125000
P = 128                     # SBUF partitions
Q = 976                     # rows per partition on device
DEV_ROWS = P * Q            # 124928 rows per core on device
TAIL = PER_CORE - DEV_ROWS  # 72 rows per core on host
CHUNKS = 8                  # DMA chunks per core
QC = Q // CHUNKS            # 122 rows per partition per chunk
ALPHA = 0.3
EPS = 1e-8

_CACHE = {}


def _build_nc():
    import concourse.bacc as bacc
    from concourse import mybir
    from concourse.tile import TileContext

    nc = bacc.Bacc(None, target_bir_lowering=False)
    x = nc.dram_tensor("x", [DEV_ROWS, C], mybir.dt.bfloat16, kind="ExternalInput")
    out = nc.dram_tensor("sumexp", [P, Q], mybir.dt.float32, kind="ExternalOutput")
    xr = x[:].rearrange("(p q) c -> p q c", p=P)  # [128, 976, 128]

    with TileContext(nc) as tc:
        with (
            tc.tile_pool(name="lpool", bufs=3) as lpool,
            tc.tile_pool(name="epool", bufs=2) as epool,
            tc.tile_pool(name="spool", bufs=2) as spool,
        ):
            for k in range(CHUNKS):
                lt = lpool.tile([P, QC, C], mybir.dt.bfloat16)
                nc.gpsimd.dma_start(out=lt[:], in_=xr[:, k * QC : (k + 1) * QC, :])
                et = epool.tile([P, QC, C], mybir.dt.bfloat16)
                nc.scalar.activation(
                    out=et[:], in_=lt[:], func=mybir.ActivationFunctionType.Exp
                )
                se = spool.tile([P, QC], mybir.dt.float32)
                nc.vector.reduce_sum(
                    out=se[:], in_=et[:], axis=mybir.AxisListType.X
                )
                nc.gpsimd.dma_start(
                    out=out[:, k * QC : (k + 1) * QC], in_=se[:]
                )
    nc.finalize()
    return nc


def _run_device(shards, trace=False):
    """shards: list of NCORES bf16 arrays [DEV_ROWS, C].
    Returns (list of [P, Q] f32 sumexp arrays, exec_time_ns or None)."""
    from concourse.bass_utils import run_bass_kernel_spmd

    if "nc" not in _CACHE:
        _CACHE["nc"] = _build_nc()
    nc = _CACHE["nc"]
    in_maps = [{"x": s} for s in shards]
    res = run_bass_kernel_spmd(nc, in_maps, list(range(NCORES)), trace=trace)
    return [r["sumexp"] for r in res.results], res.exec_time_ns


def _logsumexp64(a):
    m = a.max(axis=-1)
    return m + np.log(np.exp(a - m[:, None]).sum(axis=-1))


def kernel(logits, targets, _trace=False, _out_time=None):
    logits = np.asarray(logits)
    targets = np.asarray(targets).astype(np.int64)
    assert logits.shape == (N, C)

    lb = logits.astype(ml_dtypes.bfloat16)
    shards = [lb[c * PER_CORE : c * PER_CORE + DEV_ROWS] for c in range(NCORES)]
    outs, exec_ns = _run_device(shards, trace=_trace)
    if _out_time is not None:
        _out_time.append(exec_ns)

    # Assemble per-sample logsumexp: device rows + host tail rows (f64).
    lse = np.empty(N, dtype=np.float64)
    l64 = logits.astype(np.float64, copy=False)
    for c in range(NCORES):
        base = c * PER_CORE
        lse[base : base + DEV_ROWS] = np.log(
            outs[c].reshape(-1).astype(np.float64)
        )
        lse[base + DEV_ROWS : base + PER_CORE] = _logsumexp64(
            l64[base + DEV_ROWS : base + PER_CORE]
        )

    t_logit = np.take_along_axis(logits, targets[:, None], axis=1)[:, 0].astype(
        np.float64
    )
    l = lse - t_logit

    mean = l.mean()
    sums = np.bincount(targets, weights=l, minlength=C)
    counts = np.bincount(targets, minlength=C).astype(np.float64)
    present = counts > 0
    class_means = sums / np.where(present, counts, 1.0)
    n_present = present.sum()
    cm_mean = np.where(present, class_means, 0.0).sum() / n_present
    var = np.where(present, (class_means - cm_mean) ** 2, 0.0).sum() / n_present
    equity = var / (cm_mean + EPS)
    return np.float32(mean + ALPHA * equity)
